# revision 1
# baseline (speedup 1.0000x reference)
"""LSTM (single layer, final hidden state) on 8 Trainium2 NeuronCores.

Reference computation (per batch row b):
    gx[t] = x[t] @ w_ih.T + (b_ih + b_hh)
    g     = gx[t] + h @ w_hh.T          # [B, 4H], gate order i,f,g,o
    i,f,o = sigmoid(...), g_c = tanh(...)
    c     = f*c + i*g_c
    h     = o * tanh(c)
returns h after T steps, shape [1, B, H].

Sharding: data-parallel over batch B=256 -> 8 cores x 32. Weights replicated.

Per-core layout ("packed"): partition p = 32*j + b, where j in [0,4) indexes
an H-quarter (H index = 64*j + s, s in [0,64)) and b in [0,32) is the local
batch.  All elementwise tiles are [128, *]:
    c, h            [128, 64]   c[32j+b, s] = C[b, 64j+s]
    gate psum       [128, 256]  cols 64*q+s with q order (i, f, o, g)
Gates are produced by 4 column-tiled concurrent matmuls (tile_position
(0,32j)), accumulating 4 K-rounds: bias (K=1 ones trick), x_t (K=128),
h chunk0 (K=128), h chunk1 (K=128).  The stationary operands are the small
[K,32] transposes of x_t / h, so weight loads are cheap; the big W tiles
stream through the moving port of 4 column groups concurrently.

h -> h.T for the next step is done with 4 concurrent row+col-tiled PE
transposes ([32,64] blocks at tile_position (32j, 64*(j%2))) into one PSUM
tile, then one DVE copy to SBUF.
"""

import os
import sys

import numpy as np

B_TOT, T_FULL, I_DIM, H = 256, 1024, 128, 256
NCORES = 8
B = B_TOT // NCORES  # 32 per core
NJ = 4  # H quarters
S = H // NJ  # 64
# column order within a gate-quarter: (i, f, o, g_cell); row bases in w/b
Q_ROWBASE = (0, 256, 768, 512)


def _ensure_paths():
    for p in ("/opt/trn_rl_repo",):
        if os.path.isdir(p) and p not in sys.path:
            sys.path.append(p)


def _prep_weights(w_ih, w_hh, b_ih, b_hh):
    """Host-side permutation of weights into the packed rhs layouts."""
    wih_p = np.empty((I_DIM, NJ, 4 * S), np.float32)  # [128, 4, 256]
    whh_p = np.empty((128, 2, NJ, 4 * S), np.float32)  # [128, u, j, 256]
    bias_p = np.empty((1, NJ, 4 * S), np.float32)  # [1, 4, 256]
    bsum = (b_ih + b_hh).astype(np.float32)
    # DVE 32x32 block-transpose of packed h puts H-input index
    # 64*(k//32) + 32*u + (k%32) at partition k of lhsT column-group u.
    k = np.arange(128)
    hperm = [64 * (k // 32) + 32 * u + (k % 32) for u in range(2)]
    for q, rb in enumerate(Q_ROWBASE):
        for j in range(NJ):
            rows = slice(rb + S * j, rb + S * j + S)
            wih_p[:, j, S * q : S * q + S] = w_ih[rows, :].T
            for u in range(2):
                whh_p[:, u, j, S * q : S * q + S] = w_hh[rows, :][:, hperm[u]].T
            bias_p[0, j, S * q : S * q + S] = bsum[rows]
    ident = np.zeros((128, 32), np.float32)
    for p in range(128):
        ident[p, p % 32] = 1.0
    return wih_p, whh_p, bias_p, ident


def build_nc(T=T_FULL, TC=32, debug=False):
    """Build the per-core Bass program (SPMD: same program on all cores)."""
    _ensure_paths()
    import concourse.bacc as bacc
    import concourse.mybir as mybir
    import concourse.tile as tile
    from contextlib import ExitStack

    fp32 = mybir.dt.float32
    AF = mybir.ActivationFunctionType

    assert T % TC == 0 and TC % 8 == 0

    nc = bacc.Bacc("TRN2", target_bir_lowering=False, debug=debug)

    x_d = nc.dram_tensor("x", [B, T, I_DIM], fp32, kind="ExternalInput").ap()
    h0_d = nc.dram_tensor("h0", [B, H], fp32, kind="ExternalInput").ap()
    c0_d = nc.dram_tensor("c0", [B, H], fp32, kind="ExternalInput").ap()
    wih_d = nc.dram_tensor("wih_p", [I_DIM, NJ, 4 * S], fp32, kind="ExternalInput").ap()
    whh_d = nc.dram_tensor(
        "whh_p", [128, 2, NJ, 4 * S], fp32, kind="ExternalInput"
    ).ap()
    bias_d = nc.dram_tensor("bias_p", [1, NJ, 4 * S], fp32, kind="ExternalInput").ap()
    ident_d = nc.dram_tensor("ident", [128, 32], fp32, kind="ExternalInput").ap()
    hn_d = nc.dram_tensor("hn", [B, H], fp32, kind="ExternalOutput").ap()

    with tile.TileContext(nc) as tc, ExitStack() as ctx:
        consts = ctx.enter_context(tc.tile_pool(name="consts", bufs=1))
        states = ctx.enter_context(tc.tile_pool(name="states", bufs=1))
        lhsT_pool = ctx.enter_context(tc.tile_pool(name="lhsT", bufs=3))
        x_pool = ctx.enter_context(tc.tile_pool(name="xstream", bufs=2))
        xT_pool = ctx.enter_context(tc.tile_pool(name="xT", bufs=3))
        ew_pool = ctx.enter_context(tc.tile_pool(name="ew", bufs=3))
        g_psum = ctx.enter_context(tc.tile_pool(name="g_psum", bufs=2, space="PSUM"))
        ht_psum = ctx.enter_context(tc.tile_pool(name="ht_psum", bufs=2, space="PSUM"))
        xt_psum = ctx.enter_context(tc.tile_pool(name="xt_psum", bufs=2, space="PSUM"))

        # ---- constants ----
        wih_sb = consts.tile([I_DIM, NJ, 4 * S], fp32, name="wih_sb")
        nc.sync.dma_start(out=wih_sb, in_=wih_d)
        whh_sb = consts.tile([128, 2, NJ, 4 * S], fp32, name="whh_sb")
        nc.sync.dma_start(out=whh_sb, in_=whh_d)
        bias_sb = consts.tile([1, NJ, 4 * S], fp32, name="bias_sb")
        nc.sync.dma_start(out=bias_sb, in_=bias_d)
        ident_sb = consts.tile([128, 32], fp32, name="ident_sb")
        nc.sync.dma_start(out=ident_sb, in_=ident_d)
        ones_sb = consts.tile([1, 32], fp32, name="ones_sb")
        nc.vector.memset(ones_sb, 1.0)

        # ---- state init (packed) ----
        c_sb = states.tile([128, S], fp32, name="c_sb")
        h_sb = states.tile([128, S], fp32, name="h_sb")
        for j in range(NJ):
            nc.sync.dma_start(
                out=c_sb[32 * j : 32 * j + 32, :], in_=c0_d[:, S * j : S * j + S]
            )
            nc.sync.dma_start(
                out=h_sb[32 * j : 32 * j + 32, :], in_=h0_d[:, S * j : S * j + S]
            )

        def emit_hT():
            """DVE 32x32 block transpose of packed h -> lhsT column groups.

            hv[32J+y, 32u+x] = h[32J+x, 32u+y] = H[x, 64J+32u+y]; so
            hv[:, 32u:32u+32] is a [K=128, M=32] stationary operand whose
            K-rows enumerate H-inputs in the order 64*(k//32)+32u+(k%32) —
            whh_p is host-permuted to match.
            """
            hT = lhsT_pool.tile([128, 2 * 32], fp32, name="hT")
            nc.vector.transpose(out=hT, in_=h_sb)
            return hT

        hT = emit_hT()

        n_chunks = T // TC
        for ch in range(n_chunks):
            x_sb = x_pool.tile([B, TC, I_DIM], fp32, name="x_sb")
            nc.sync.dma_start(out=x_sb, in_=x_d[:, ch * TC : (ch + 1) * TC, :])
            xT_tiles = []
            for g8 in range(TC // 8):
                xt_ps = xt_psum.tile([128, 8 * 32], fp32, name="xt_ps")
                for u in range(8):
                    nc.tensor.transpose(
                        out=xt_ps[:, 32 * u : 32 * u + 32],
                        in_=x_sb[:, g8 * 8 + u, :],
                        identity=ident_sb[0:32, :],
                        tile_position=(0, 0),
                    )
                xT_sb = xT_pool.tile([128, 8 * 32], fp32, name="xT_sb")
                nc.vector.tensor_copy(out=xT_sb, in_=xt_ps)
                xT_tiles.append(xT_sb)

            for u in range(TC):
                t = ch * TC + u
                xT_sl = xT_tiles[u // 8][:, 32 * (u % 8) : 32 * (u % 8) + 32]
                g_ps = g_psum.tile([128, 4 * S], fp32, name="g_ps")
                # round-major emission for cross-column-group concurrency
                for rnd in range(4):
                    for j in range(NJ):
                        out = g_ps[32 * j : 32 * j + 32, :]
                        kw = dict(tile_position=(0, 32 * j), skip_group_check=True)
                        if rnd == 0:
                            nc.tensor.matmul(
                                out, ones_sb, bias_sb[0:1, j, :],
                                start=True, stop=False, **kw,
                            )
                        elif rnd == 1:
                            nc.tensor.matmul(
                                out, xT_sl, wih_sb[:, j, :],
                                start=False, stop=False, **kw,
                            )
                        else:
                            u = rnd - 2
                            nc.tensor.matmul(
                                out,
                                hT[:, 32 * u : 32 * u + 32],
                                whh_sb[:, u, j, :],
                                start=False, stop=(rnd == 3), **kw,
                            )
                # gates: cols [0:64]=i [64:128]=f [128:192]=o [192:256]=g_cell
                sig = ew_pool.tile([128, 3 * S], fp32, name="sig")
                nc.scalar.activation(sig, g_ps[:, 0 : 3 * S], AF.Sigmoid)
                tg = ew_pool.tile([128, S], fp32, name="tg")
                nc.scalar.activation(tg, g_ps[:, 3 * S : 4 * S], AF.Tanh)
                t1 = ew_pool.tile([128, S], fp32, name="t1")
                nc.vector.tensor_mul(t1, sig[:, 0:S], tg)
                nc.vector.tensor_mul(c_sb, sig[:, S : 2 * S], c_sb)
                nc.vector.tensor_add(c_sb, c_sb, t1)
                tcc = ew_pool.tile([128, S], fp32, name="tcc")
                nc.scalar.activation(tcc, c_sb, AF.Tanh)
                nc.vector.tensor_mul(h_sb, sig[:, 2 * S : 3 * S], tcc)
                if t < T - 1:
                    hT = emit_hT()

        # ---- write back final h (unpack) ----
        for j in range(NJ):
            nc.sync.dma_start(
                out=hn_d[:, S * j : S * j + S], in_=h_sb[32 * j : 32 * j + 32, :]
            )

    nc.compile()
    return nc


def _shard_inputs(x, h0, c0, w_ih, w_hh, b_ih, b_hh, T=T_FULL):
    wih_p, whh_p, bias_p, ident = _prep_weights(
        np.asarray(w_ih, np.float32),
        np.asarray(w_hh, np.float32),
        np.asarray(b_ih, np.float32),
        np.asarray(b_hh, np.float32),
    )
    x = np.asarray(x, np.float32)
    h0 = np.asarray(h0, np.float32)
    c0 = np.asarray(c0, np.float32)
    in_maps = []
    for k in range(NCORES):
        bs = slice(B * k, B * (k + 1))
        in_maps.append(
            {
                "x": np.ascontiguousarray(x[bs, :T, :]),
                "h0": np.ascontiguousarray(h0[0, bs, :]),
                "c0": np.ascontiguousarray(c0[0, bs, :]),
                "wih_p": wih_p,
                "whh_p": whh_p,
                "bias_p": bias_p,
                "ident": ident,
            }
        )
    return in_maps


_NC_CACHE = {}


def run_hw(x, h0, c0, w_ih, w_hh, b_ih, b_hh, T=T_FULL, TC=32, trace=False):
    _ensure_paths()
    from concourse.bass_utils import run_bass_kernel_spmd

    key = (T, TC)
    if key not in _NC_CACHE:
        _NC_CACHE[key] = build_nc(T=T, TC=TC)
    nc = _NC_CACHE[key]
    in_maps = _shard_inputs(x, h0, c0, w_ih, w_hh, b_ih, b_hh, T=T)
    res = run_bass_kernel_spmd(nc, in_maps, list(range(NCORES)), trace=trace)
    hn = np.stack([res.results[k]["hn"] for k in range(NCORES)], axis=0)
    return hn.reshape(1, B_TOT, H), res


def kernel(x, h0, c0, w_ih, w_hh, b_ih, b_hh):
    out, _ = run_hw(x, h0, c0, w_ih, w_hh, b_ih, b_hh)
    return out.astype(np.float32)


def _np_reference(x, h0, c0, w_ih, w_hh, b_ih, b_hh, T=None):
    """Numpy oracle for development (matches reference.py)."""
    x = np.asarray(x, np.float64)
    if T is not None:
        x = x[:, :T, :]
    h = np.asarray(h0, np.float64)[0]
    c = np.asarray(c0, np.float64)[0]
    gx = np.einsum("bti,gi->tbg", x, np.asarray(w_ih, np.float64)) + (
        np.asarray(b_ih, np.float64) + np.asarray(b_hh, np.float64)
    )
    W = np.asarray(w_hh, np.float64)

    def sg(v):
        return 1.0 / (1.0 + np.exp(-v))

    for t in range(x.shape[1]):
        g = gx[t] + h @ W.T
        i = sg(g[:, 0:256])
        f = sg(g[:, 256:512])
        gg = np.tanh(g[:, 512:768])
        o = sg(g[:, 768:1024])
        c = f * c + i * gg
        h = o * np.tanh(c)
    return h[None].astype(np.float32)



# revision 7
# speedup vs baseline: 1.5513x; 1.5513x over previous
"""LSTM (single layer, final hidden state) on 8 Trainium2 NeuronCores.

Reference computation (per batch row b):
    gx[t] = x[t] @ w_ih.T + (b_ih + b_hh)
    g     = gx[t] + h @ w_hh.T          # [B, 4H], gate order i,f,g,o
    i,f,o = sigmoid(...), g_c = tanh(...)
    c     = f*c + i*g_c
    h     = o * tanh(c)
returns h after T steps, shape [1, B, H].

Sharding: data-parallel over batch B=256 -> 8 cores x 32. Weights replicated.

Per-core layout ("packed"): partition p = 32*j + b, where j in [0,4) indexes
an H-quarter (H index = 64*j + s, s in [0,64)) and b in [0,32) is the local
batch.  Gate-column order per quarter (host-permuted): [g2 | f | i | o],
where the g-gate rows of W/bias are pre-scaled by 2 so that
tanh(g) = 2*sigmoid(2g) - 1 can be recovered from a plain sigmoid.

Per step (all matmuls fp32r, 1 cycle/row at N=256, 4 column groups
concurrent via tile_position):
  PE   : 4 accumulation rounds into a PSUM bank: x_t (start), bias,
         h chunk0, h chunk1 (stop).  Stationaries are small [K,32] tiles
         (xT slice / hT halves / ones); W streams through the moving port.
  ACT  : sigmoid over PSUM cols 0:192 (g2,f,i) -> GB[:, 64:256];
         sigmoid over cols 192:256 (o) -> GB[:, 256:320] (off critical path)
  DVE  : FCIT custom op  w = [f|i] * ([c|s]*(1+ge) - ge), ge=(k>=64)
           (i.e. w = [f*c | i*(2s-1)] = [f*c | i*tanh(g)])
         TT add  c' = w[:,0:64] + w[:,64:128]  (in-place into GB[:,0:64])
  ACT  : tanh(c') -> tcc
  DVE  : h = sigma_o * tcc;  hT = 32x32-block transpose of h (lhsT for the
         next step's h rounds; whh host-permuted to match).

x is pre-transposed on the host to xT [I, T, B] so the per-step lhsT
slices DMA straight into SBUF (no PE transposes / PSUM evictions).
"""

import os
import sys

import numpy as np

B_TOT, T_FULL, I_DIM, H = 256, 1024, 128, 256
NCORES = 8
B = B_TOT // NCORES  # 32 per core
NJ = 4  # H quarters
S = H // NJ  # 64
# gate-column order within a quarter: (g_cell, f, i, o); row bases in w/b
# (PyTorch row order is i,f,g,o at 256 rows each)
Q_ROWBASE = (512, 256, 0, 768)
Q_SCALE = (2.0, 1.0, 1.0, 1.0)


def _ensure_paths():
    for p in ("/opt/trn_rl_repo",):
        if os.path.isdir(p) and p not in sys.path:
            sys.path.append(p)


_FCIT = None


def _get_fcit():
    """Register (once) and return the fused f*c / i*tanh-from-sigmoid DVE op.

    out[p,k] = in0[p,k] * (in1[p,k]*(1+ge) - ge),  ge = (k >= imm2)
    """
    global _FCIT
    if _FCIT is not None:
        return _FCIT
    import concourse.dve_ops as dve_ops
    from concourse.dve_spec import C2, Idx, One, Spec, Src0, Src1, lower
    from concourse.dve_uop import DveOpSpec

    name = "LSTM_FCIT_V1"
    for op in dve_ops.OPS:
        if op.name == name:
            _FCIT = op
            return op

    ge = Idx >= C2
    body = Src0 * (Src1 * (One + ge) - ge)

    def ref(in0, in1, s0, s1, imm2):
        n = in0.shape[-1]
        g = (np.arange(n) >= imm2).astype(np.float32)
        return (
            in0.astype(np.float32) * (in1.astype(np.float32) * (1.0 + g) - g)
        ).astype(np.float32)

    spec = Spec(body=body, reference=ref)
    opcode = dve_ops._CUSTOM_DVE_ROW_BASE + len(dve_ops.OPS)
    assert opcode < 0x20
    shas = {}
    for ver in ("v3", "v4"):
        ds = DveOpSpec(
            name=name, opcode=opcode, uops=lower(spec, ver=ver), rd1_en=True
        )
        shas[ver] = ds.sha(ver)
    op = dve_ops.DveOp(name=name, spec=spec, subdim=False, uops_sha=shas)
    dve_ops.OPS.append(op)
    dve_ops._SUB_OPCODE_FOR_NAME[name] = opcode
    dve_ops.CUSTOM_DVE_SPECS[name] = spec
    _FCIT = op
    return op


def _to_bf16(a):
    import ml_dtypes

    return np.asarray(a, np.float32).astype(ml_dtypes.bfloat16)


def _prep_weights(w_ih, w_hh, b_ih, b_hh):
    """Host-side permutation of weights into the packed rhs layouts."""
    wih_p = np.empty((I_DIM, NJ, 4 * S), np.float32)  # [128, 4, 256]
    whh_p = np.empty((128, 2, NJ, 4 * S), np.float32)  # [128, u, j, 256]
    bias_p = np.empty((1, NJ, 4 * S), np.float32)  # [1, 4, 256]
    bsum = (b_ih + b_hh).astype(np.float32)
    # DVE 32x32 block-transpose of packed h puts H-input index
    # 64*(k//32) + 32*u + (k%32) at partition k of lhsT column-group u.
    k = np.arange(128)
    hperm = [64 * (k // 32) + 32 * u + (k % 32) for u in range(2)]
    for q, (rb, sc) in enumerate(zip(Q_ROWBASE, Q_SCALE)):
        for j in range(NJ):
            rows = slice(rb + S * j, rb + S * j + S)
            wih_p[:, j, S * q : S * q + S] = sc * w_ih[rows, :].T
            for u in range(2):
                whh_p[:, u, j, S * q : S * q + S] = sc * w_hh[rows, :][:, hperm[u]].T
            bias_p[0, j, S * q : S * q + S] = sc * bsum[rows]
    return wih_p, whh_p, bias_p


def build_nc(T=T_FULL, debug=False):
    """Build the per-core Bass program (SPMD: same program on all cores)."""
    _ensure_paths()
    import concourse.bacc as bacc
    import concourse.mybir as mybir
    import concourse.tile as tile
    from contextlib import ExitStack

    fp32 = mybir.dt.float32
    bf16 = mybir.dt.bfloat16
    AF = mybir.ActivationFunctionType
    FCIT = _get_fcit()

    TG = 8  # steps per xT DMA group
    assert T % TG == 0

    nc = bacc.Bacc("TRN2", target_bir_lowering=False, debug=debug)

    xT_d = nc.dram_tensor("xT", [I_DIM, T, B], bf16, kind="ExternalInput").ap()
    h0_d = nc.dram_tensor("h0", [B, H], bf16, kind="ExternalInput").ap()
    c0_d = nc.dram_tensor("c0", [B, H], fp32, kind="ExternalInput").ap()
    wih_d = nc.dram_tensor("wih_p", [I_DIM, NJ, 4 * S], bf16, kind="ExternalInput").ap()
    whh_d = nc.dram_tensor(
        "whh_p", [128, 2, NJ, 4 * S], bf16, kind="ExternalInput"
    ).ap()
    bias_d = nc.dram_tensor("bias_p", [1, NJ, 4 * S], bf16, kind="ExternalInput").ap()
    ones_d = nc.dram_tensor("ones", [1, 32], bf16, kind="ExternalInput").ap()
    hn_d = nc.dram_tensor("hn", [B, H], fp32, kind="ExternalOutput").ap()

    with tile.TileContext(nc) as tc, ExitStack() as ctx:
        consts = ctx.enter_context(tc.tile_pool(name="consts", bufs=1))
        states = ctx.enter_context(tc.tile_pool(name="states", bufs=1))
        lhsT_pool = ctx.enter_context(tc.tile_pool(name="lhsT", bufs=3))
        xT_pool = ctx.enter_context(tc.tile_pool(name="xT", bufs=4))
        ew_pool = ctx.enter_context(tc.tile_pool(name="ew", bufs=3))
        g_psum = ctx.enter_context(tc.tile_pool(name="g_psum", bufs=4, space="PSUM"))

        # ---- constants ----
        wih_sb = consts.tile([I_DIM, NJ, 4 * S], bf16, name="wih_sb")
        nc.sync.dma_start(out=wih_sb, in_=wih_d)
        whh_sb = consts.tile([128, 2, NJ, 4 * S], bf16, name="whh_sb")
        nc.sync.dma_start(out=whh_sb, in_=whh_d)
        bias_sb = consts.tile([1, NJ, 4 * S], bf16, name="bias_sb")
        nc.sync.dma_start(out=bias_sb, in_=bias_d)
        ones_sb = consts.tile([1, 32], bf16, name="ones_sb")
        nc.sync.dma_start(out=ones_sb, in_=ones_d)

        # ---- state ----
        # GB: [c (0:64) | sig2g (64:128) | sig_f (128:192) | sig_i (192:256)
        #      | sig_o (256:320)]
        GB = states.tile([128, 5 * S], fp32, name="GB")
        for j in range(NJ):
            nc.sync.dma_start(
                out=GB[32 * j : 32 * j + 32, 0:S], in_=c0_d[:, S * j : S * j + S]
            )
        h_init = states.tile([128, S], bf16, name="h_init")
        for j in range(NJ):
            nc.sync.dma_start(
                out=h_init[32 * j : 32 * j + 32, :], in_=h0_d[:, S * j : S * j + S]
            )

        def emit_hT(h_tile):
            """DVE 32x32 block transpose of packed h -> lhsT column groups."""
            hT = lhsT_pool.tile([128, 2 * 32], bf16, name="hT")
            nc.vector.transpose(out=hT, in_=h_tile)
            return hT

        hT = emit_hT(h_init)

        xT_tiles = {}

        def load_xT(tg):
            xt = xT_pool.tile([I_DIM, TG, B], bf16, name="xt")
            nc.sync.dma_start(out=xt, in_=xT_d[:, tg * TG : (tg + 1) * TG, :])
            xT_tiles[tg] = xt

        for tg in range(min(2, T // TG)):
            load_xT(tg)

        for t in range(T):
            if t % TG == 0 and t // TG + 2 < T // TG:
                load_xT(t // TG + 2)

            xT_sl = xT_tiles[t // TG][:, t % TG, :]
            g_ps = g_psum.tile([128, 4 * S], fp32, name="g_ps", padded_shape=[128, 512])
            # round-major emission for cross-column-group concurrency
            for rnd in range(4):
                for j in range(NJ):
                    out = g_ps[32 * j : 32 * j + 32, :]
                    kw = dict(tile_position=(0, 32 * j), skip_group_check=True)
                    if rnd == 0:
                        nc.tensor.matmul(
                            out, xT_sl, wih_sb[:, j, :],
                            start=True, stop=False, **kw,
                        )
                    elif rnd == 1:
                        nc.tensor.matmul(
                            out, ones_sb, bias_sb[0:1, j, :],
                            start=False, stop=False, **kw,
                        )
                    else:
                        u = rnd - 2
                        nc.tensor.matmul(
                            out,
                            hT[:, 32 * u : 32 * u + 32],
                            whh_sb[:, u, j, :],
                            start=False, stop=(rnd == 3), **kw,
                        )
            # PSUM cols: [g2 (0:64) | f (64:128) | i (128:192) | o (192:256)]
            nc.scalar.activation(GB[:, S : 4 * S], g_ps[:, 0 : 3 * S], AF.Sigmoid)
            nc.scalar.activation(GB[:, 4 * S : 5 * S], g_ps[:, 3 * S : 4 * S], AF.Sigmoid)
            # w = [f*c | i*(2*sig2g - 1)]
            w = ew_pool.tile([128, 2 * S], fp32, name="w")
            nc.vector._custom_dve(
                FCIT, out=w, in0=GB[:, 2 * S : 4 * S], in1=GB[:, 0 : 2 * S],
                imm2=float(S),
            )
            nc.vector.tensor_add(GB[:, 0:S], w[:, 0:S], w[:, S : 2 * S])
            tcc = ew_pool.tile([128, S], fp32, name="tcc")
            nc.scalar.activation(tcc, GB[:, 0:S], AF.Tanh)
            h = ew_pool.tile([128, S], bf16, name="h")
            nc.vector.tensor_mul(h, GB[:, 4 * S : 5 * S], tcc)
            if t < T - 1:
                hT = emit_hT(h)
            else:
                h_out = states.tile([128, S], fp32, name="h_out")
                nc.vector.tensor_mul(h_out, GB[:, 4 * S : 5 * S], tcc)

        # ---- write back final h (unpack) ----
        for j in range(NJ):
            nc.sync.dma_start(
                out=hn_d[:, S * j : S * j + S], in_=h_out[32 * j : 32 * j + 32, :]
            )

    nc.compile()
    return nc


def _shard_inputs(x, h0, c0, w_ih, w_hh, b_ih, b_hh, T=T_FULL):
    wih_p, whh_p, bias_p = _prep_weights(
        np.asarray(w_ih, np.float32),
        np.asarray(w_hh, np.float32),
        np.asarray(b_ih, np.float32),
        np.asarray(b_hh, np.float32),
    )
    x = np.asarray(x, np.float32)
    h0 = np.asarray(h0, np.float32)
    c0 = np.asarray(c0, np.float32)
    in_maps = []
    for k in range(NCORES):
        bs = slice(B * k, B * (k + 1))
        # xT: [I, T, B] per-core slice, host-transposed, bf16
        xT = _to_bf16(np.ascontiguousarray(x[bs, :T, :].transpose(2, 1, 0)))
        in_maps.append(
            {
                "xT": xT,
                "h0": _to_bf16(np.ascontiguousarray(h0[0, bs, :])),
                "c0": np.ascontiguousarray(c0[0, bs, :]),
                "wih_p": _to_bf16(wih_p),
                "whh_p": _to_bf16(whh_p),
                "bias_p": _to_bf16(bias_p),
                "ones": _to_bf16(np.ones((1, 32), np.float32)),
            }
        )
    return in_maps


_NC_CACHE = {}


def run_hw(x, h0, c0, w_ih, w_hh, b_ih, b_hh, T=T_FULL, trace=False):
    _ensure_paths()
    from concourse.bass_utils import run_bass_kernel_spmd

    key = (T,)
    if key not in _NC_CACHE:
        _NC_CACHE[key] = build_nc(T=T)
    nc = _NC_CACHE[key]
    in_maps = _shard_inputs(x, h0, c0, w_ih, w_hh, b_ih, b_hh, T=T)
    res = run_bass_kernel_spmd(nc, in_maps, list(range(NCORES)), trace=trace)
    hn = np.stack([res.results[k]["hn"] for k in range(NCORES)], axis=0)
    return hn.reshape(1, B_TOT, H), res


def kernel(x, h0, c0, w_ih, w_hh, b_ih, b_hh):
    out, _ = run_hw(x, h0, c0, w_ih, w_hh, b_ih, b_hh)
    return out.astype(np.float32)


def _np_reference(x, h0, c0, w_ih, w_hh, b_ih, b_hh, T=None):
    """Numpy oracle for development (matches reference.py)."""
    x = np.asarray(x, np.float64)
    if T is not None:
        x = x[:, :T, :]
    h = np.asarray(h0, np.float64)[0]
    c = np.asarray(c0, np.float64)[0]
    gx = np.einsum("bti,gi->tbg", x, np.asarray(w_ih, np.float64)) + (
        np.asarray(b_ih, np.float64) + np.asarray(b_hh, np.float64)
    )
    W = np.asarray(w_hh, np.float64)

    def sg(v):
        return 1.0 / (1.0 + np.exp(-v))

    for t in range(x.shape[1]):
        g = gx[t] + h @ W.T
        i = sg(g[:, 0:256])
        f = sg(g[:, 256:512])
        gg = np.tanh(g[:, 512:768])
        o = sg(g[:, 768:1024])
        c = f * c + i * gg
        h = o * np.tanh(c)
    return h[None].astype(np.float32)
